# revision 7
# baseline (speedup 1.0000x reference)
"""VQ color-lookup kernel for Trainium2 (8 NeuronCores, data-parallel over batch).

Problem: z [8,3,256,256] f32, color_table [512,3] f32.
  zl = z transposed to [B,H,W,3]; nearest color (squared L2) per pixel;
  out = quantized colors back in [B,3,H,W]; loss = 11*mean((z_q-zl)^2).

Per-core work (core b handles batch item b, N=65536 pixels):
  scores s[n,k] = <z_n, t_k> - 0.5*|t_k|^2  (argmax_k s = argmin_k d^2)
  computed by the PE as one K=4 fp32 matmul per 128-pixel tile
  (4th contraction row = ones against the -0.5*|t|^2 bias row).
  DVE max8 + max_index give the winning score + color index per pixel.
  z_q rows are fetched with one small indirect-DMA gather per tile
  (128 rows x 16B), overlapped with the DVE-bound main loop.
  Loss uses d2_min = |z|^2 - 2*s_max, accumulated per partition on ACT.
"""
import numpy as np

import concourse.bass as bass
import concourse.mybir as mybir
from concourse.tile import TileContext
from concourse.vector_clock import ScopedClock
from concourse.bass_utils import run_bass_kernel_spmd

B, C, H, W = 8, 3, 256, 256
N = H * W            # pixels per core (batch-sharded, one image per core)
K = 512              # colors
NB = 8               # super-blocks per core
TL = 64              # pixel tiles per super-block
BLK = N // NB        # pixels per super-block (8192)
NT = NB * TL         # total 128-pixel tiles per core (512)
F32 = mybir.dt.float32
BETA = 10.0


class TC(TileContext):
    """TileContext that splits sem waits one-per-instruction
    (this walrus build rejects >1 sync wait on a single instruction)."""

    def _split_waits(self, inst):
        si = getattr(inst, "sync_info", None)
        if not si or not si.on_wait or len(si.on_wait) <= 1:
            return
        if inst.engine == mybir.EngineType.Unassigned:
            return
        waits = list(si.on_wait)
        inst.sync_info = mybir.SyncInfo(
            on_wait=[waits[-1]], on_update=list(si.on_update or [])
        )
        for w in waits[:-1]:
            nop = mybir.InstNoOp(
                name=self.nc.get_next_instruction_name(), ins=[], outs=[]
            )
            nop.engine = inst.engine
            nop.sync_info = mybir.SyncInfo(on_wait=[w], on_update=[])
            self._add_instruction(nop)

    def _commit_instruction(self, inst, lazy_reg_writes: bool = True):
        self._split_waits(inst)
        super()._commit_instruction(inst, lazy_reg_writes)

    def _drain_and_barrier(self, tick_clock, wait_clock):
        nc = self.nc
        drain_inst = nc.sync.drain()
        wait_clock.add_sem_waits(
            drain_inst.ins, ScopedClock({None: tick_clock.global_clock})
        )
        si = drain_inst.ins.sync_info
        waits = list(si.on_wait) if si and si.on_wait else []
        if len(waits) > 1:
            drain_inst.ins.sync_info = mybir.SyncInfo(on_wait=[waits[0]], on_update=[])
            for w in waits[1:]:
                nop = nc.sync.nop(nofuse=True, hint="drain_split")
                nop.ins.sync_info = mybir.SyncInfo(on_wait=[w], on_update=[])
        nc.all_engine_barrier()
        assert self.sems is not None
        popped = nc._tile_sem_poison_stack.pop()
        assert popped is self._sem_poison
        nc.clear_and_free_semaphores(list(self.sems.allocated().values()))
        nc.all_engine_barrier()


def build_kernel():
    nc = bass.Bass("TRN2", target_bir_lowering=False, debug=False, num_devices=8)
    z4 = nc.declare_dram_parameter("z4", [4, N], F32, isOutput=False)
    tba = nc.declare_dram_parameter("tba", [4, K], F32, isOutput=False)
    tb4 = nc.declare_dram_parameter("tb4", [K, 4], F32, isOutput=False)
    o = nc.declare_dram_parameter("o", [3, N], F32, isOutput=True)
    st = nc.declare_dram_parameter("st", [128, 2], F32, isOutput=True)

    with TC(nc) as tc:
        with (
            tc.tile_pool(name="const", bufs=1) as cpool,
            tc.tile_pool(name="zin", bufs=2) as zpool,
            tc.tile_pool(name="psum", bufs=6, space="PSUM") as pspool,
        ):
            tba_s = cpool.tile([4, K], F32, tag="tba")
            nc.sync.dma_start(out=tba_s[:], in_=tba[:])

            smax_all = cpool.tile([128, 8 * NT], F32, tag="smax")
            idx_all = cpool.tile([128, 8 * NT], mybir.dt.uint32, tag="idx")
            stats = cpool.tile([128, 2], F32, tag="stats")
            zq4 = cpool.tile([128, 4 * NT], F32, tag="zq4")
            outc = cpool.tile([128, 3 * NT], F32, tag="outc")

            # |z|^2 per-partition sums: z4 rows 0..2 flat = 196608 = 128*1536
            zsq = cpool.tile([128, 1536], F32, tag="zsq")
            sqd = cpool.tile([128, 1536], F32, tag="sqd")
            nc.sync.dma_start(
                out=zsq[:],
                in_=z4[0:3, :].rearrange("c n -> (c n)").rearrange(
                    "(p f) -> p f", p=128
                ),
            )
            nc.scalar.activation(
                out=sqd[:], in_=zsq[:],
                func=mybir.ActivationFunctionType.Square,
                accum_out=stats[:, 1:2],
            )

            # main loop: scores -> max -> argmax -> per-tile gather
            for s in range(NB):
                zb = zpool.tile([4, BLK], F32, tag="zb")
                nc.sync.dma_start(out=zb[:], in_=z4[:, s * BLK : (s + 1) * BLK])
                # pixel(p; s, t) = s*BLK + p*TL + t
                zb_r = zb[:].rearrange("c (p t) -> c t p", t=TL)
                for t in range(TL):
                    j = s * TL + t
                    ps = pspool.tile([128, K], F32, tag="ps")
                    nc.tensor.matmul(
                        out=ps[:], lhsT=zb_r[:, t, :], rhs=tba_s[:],
                        start=True, stop=True,
                    )
                    nc.vector.max(smax_all[:, j * 8 : (j + 1) * 8], ps[:])
                    nc.vector.max_index(
                        idx_all[:, j * 8 : (j + 1) * 8],
                        smax_all[:, j * 8 : (j + 1) * 8],
                        ps[:],
                    )
                    nc.gpsimd.indirect_dma_start(
                        out=zq4[:, 4 * j : 4 * (j + 1)],
                        out_offset=None,
                        in_=tb4[:],
                        in_offset=bass.IndirectOffsetOnAxis(
                            ap=idx_all[:, 8 * j : 8 * j + 1], axis=0
                        ),
                    )

            # sum of per-pixel max scores (col 0 of each 8-group), per partition
            smax_v = smax_all[:].rearrange("p (j e) -> p j e", e=8)[:, :, 0]
            scp = cpool.tile([128, NT], F32, tag="scp")
            nc.scalar.activation(
                out=scp[:], in_=smax_v,
                func=mybir.ActivationFunctionType.Copy,
                accum_out=stats[:, 0:1],
            )
            nc.sync.dma_start(out=st[:], in_=stats[:])

            # de-interleave channels and store output planes
            zq_v = zq4[:].rearrange("p (j c) -> p j c", c=4)
            for c in range(3):
                oc = outc[:, c * NT : (c + 1) * NT]
                nc.vector.tensor_copy(out=oc, in_=zq_v[:, :, c])
                nc.sync.dma_start(
                    out=o[c, :].rearrange("(s p t) -> p s t", s=NB, p=128, t=TL),
                    in_=oc.rearrange("p (s t) -> p s t", s=NB, t=TL),
                )
    return nc


_NC_CACHE = None
TRACE = False          # set True to capture an NTFF profile on the next call
LAST_EXEC_NS = None    # HW exec time from the last traced run
LAST_PROFILE = None    # profile json path/object from the last traced run


def _get_nc():
    global _NC_CACHE
    if _NC_CACHE is None:
        _NC_CACHE = build_kernel()
    return _NC_CACHE


def kernel(z: np.ndarray, color_table: np.ndarray):
    z = np.ascontiguousarray(np.asarray(z, dtype=np.float32))
    table = np.ascontiguousarray(np.asarray(color_table, dtype=np.float32))
    assert z.shape == (B, C, H, W) and table.shape == (K, 3)

    t2 = (table.astype(np.float64) ** 2).sum(-1).astype(np.float32)
    tba = np.concatenate([table.T, (-0.5 * t2)[None, :]], axis=0).astype(np.float32)
    tb4 = np.ascontiguousarray(
        np.concatenate([table, np.zeros((K, 1), np.float32)], axis=1)
    )

    ones = np.ones((1, N), np.float32)
    in_maps = []
    for b in range(B):
        z4 = np.ascontiguousarray(
            np.concatenate([z[b].reshape(3, N), ones], axis=0)
        )
        in_maps.append({"z4": z4, "tba": tba, "tb4": tb4})

    nc = _get_nc()
    global LAST_EXEC_NS, LAST_PROFILE
    res = run_bass_kernel_spmd(nc, in_maps, list(range(B)), trace=TRACE)
    if TRACE:
        LAST_EXEC_NS = res.exec_time_ns
        LAST_PROFILE = res.profile_json

    out = np.empty((B, C, H, W), np.float32)
    d2sum = 0.0
    for b in range(B):
        out[b] = res.results[b]["o"].reshape(3, H, W)
        stb = res.results[b]["st"].astype(np.float64)
        d2sum += stb[:, 1].sum() - 2.0 * stb[:, 0].sum()
    loss = np.float32((BETA + 1.0) * d2sum / (3.0 * B * N))
    return out, loss


if __name__ == "__main__":
    rng = np.random.default_rng(0)
    z = rng.standard_normal((B, C, H, W)).astype(np.float32)
    tbl = rng.random((K, 3), dtype=np.float32)
    out, loss = kernel(z=z, color_table=tbl)
    print("out", out.shape, "loss", loss)


# revision 11
# speedup vs baseline: 1.0175x; 1.0175x over previous
"""VQ color-lookup kernel for Trainium2 (8 NeuronCores, data-parallel over batch).

Problem: z [8,3,256,256] f32, color_table [512,3] f32.
  zl = z transposed to [B,H,W,3]; nearest color (squared L2) per pixel;
  out = quantized colors back in [B,3,H,W]; loss = 11*mean((z_q-zl)^2).

Per-core work (core b handles batch item b, N=65536 pixels):
  scores s[n,k] = <z_n, t_k> - 0.5*|t_k|^2  (argmax_k s = argmin_k d^2)
  computed by the PE as one K=4 fp32 matmul per 128-pixel tile
  (4th contraction row = ones against the -0.5*|t|^2 bias row).
  DVE max8 + max_index give the winning score + color index per pixel.
  z_q rows are fetched with one small indirect-DMA gather per tile
  (128 rows x 16B), overlapped with the DVE-bound main loop.
  Loss uses d2_min = |z|^2 - 2*s_max, accumulated per partition on ACT.
"""
import numpy as np

import concourse.bass as bass
import concourse.mybir as mybir
from concourse.tile import TileContext
from concourse.vector_clock import ScopedClock
from concourse.bass_utils import run_bass_kernel_spmd

B, C, H, W = 8, 3, 256, 256
N = H * W            # pixels per core (batch-sharded, one image per core)
K = 512              # colors
NB = 8               # super-blocks per core
TL = 64              # pixel tiles per super-block
BLK = N // NB        # pixels per super-block (8192)
NT = NB * TL         # total 128-pixel tiles per core (512)
F32 = mybir.dt.float32
BETA = 10.0


class TC(TileContext):
    """TileContext that splits sem waits one-per-instruction
    (this walrus build rejects >1 sync wait on a single instruction)."""

    def _split_waits(self, inst):
        si = getattr(inst, "sync_info", None)
        if not si or not si.on_wait or len(si.on_wait) <= 1:
            return
        if inst.engine == mybir.EngineType.Unassigned:
            return
        waits = list(si.on_wait)
        inst.sync_info = mybir.SyncInfo(
            on_wait=[waits[-1]], on_update=list(si.on_update or [])
        )
        for w in waits[:-1]:
            nop = mybir.InstNoOp(
                name=self.nc.get_next_instruction_name(), ins=[], outs=[]
            )
            nop.engine = inst.engine
            nop.sync_info = mybir.SyncInfo(on_wait=[w], on_update=[])
            self._add_instruction(nop)

    def _commit_instruction(self, inst, lazy_reg_writes: bool = True):
        self._split_waits(inst)
        super()._commit_instruction(inst, lazy_reg_writes)

    def _drain_and_barrier(self, tick_clock, wait_clock):
        nc = self.nc
        drain_inst = nc.sync.drain()
        wait_clock.add_sem_waits(
            drain_inst.ins, ScopedClock({None: tick_clock.global_clock})
        )
        si = drain_inst.ins.sync_info
        waits = list(si.on_wait) if si and si.on_wait else []
        if len(waits) > 1:
            drain_inst.ins.sync_info = mybir.SyncInfo(on_wait=[waits[0]], on_update=[])
            for w in waits[1:]:
                nop = nc.sync.nop(nofuse=True, hint="drain_split")
                nop.ins.sync_info = mybir.SyncInfo(on_wait=[w], on_update=[])
        nc.all_engine_barrier()
        assert self.sems is not None
        popped = nc._tile_sem_poison_stack.pop()
        assert popped is self._sem_poison
        nc.clear_and_free_semaphores(list(self.sems.allocated().values()))
        nc.all_engine_barrier()


def build_kernel():
    BF16 = mybir.dt.bfloat16
    nc = bass.Bass("TRN2", target_bir_lowering=False, debug=False, num_devices=8)
    z4 = nc.declare_dram_parameter("z4", [4, N], F32, isOutput=False)
    z4h = nc.declare_dram_parameter("z4h", [4, N], BF16, isOutput=False)
    z4l = nc.declare_dram_parameter("z4l", [4, N], BF16, isOutput=False)
    tbah = nc.declare_dram_parameter("tbah", [4, K], BF16, isOutput=False)
    tbal = nc.declare_dram_parameter("tbal", [4, K], BF16, isOutput=False)
    tb4 = nc.declare_dram_parameter("tb4", [K, 4], F32, isOutput=False)
    o = nc.declare_dram_parameter("o", [3, N], F32, isOutput=True)
    st = nc.declare_dram_parameter("st", [128, 2], F32, isOutput=True)

    with TC(nc) as tc:
        with (
            tc.tile_pool(name="const", bufs=1) as cpool,
            tc.tile_pool(name="zin", bufs=2) as zpool,
            tc.tile_pool(name="psum", bufs=2, space="PSUM") as pspool,
        ):
            tba_h = cpool.tile([4, K], BF16, tag="tbah")
            nc.sync.dma_start(out=tba_h[:], in_=tbah[:])
            tba_l = cpool.tile([4, K], BF16, tag="tbal")
            nc.sync.dma_start(out=tba_l[:], in_=tbal[:])

            smax_all = cpool.tile([128, 8 * NT], F32, tag="smax")
            idx_all = cpool.tile([128, 8 * NT], mybir.dt.uint32, tag="idx")
            stats = cpool.tile([128, 2], F32, tag="stats")
            zq4 = cpool.tile([128, 4 * NT], F32, tag="zq4")
            outc = cpool.tile([128, 3 * NT], F32, tag="outc")

            # |z|^2 per-partition sums: z4 rows 0..2 flat = 196608 = 128*1536
            zsq = cpool.tile([128, 1536], F32, tag="zsq")
            sqd = cpool.tile([128, 1536], F32, tag="sqd")
            nc.sync.dma_start(
                out=zsq[:],
                in_=z4[0:3, :].rearrange("c n -> (c n)").rearrange(
                    "(p f) -> p f", p=128
                ),
            )
            nc.scalar.activation(
                out=sqd[:], in_=zsq[:],
                func=mybir.ActivationFunctionType.Square,
                accum_out=stats[:, 1:2],
            )

            # main loop: scores (bf16 hi/lo split, fp32 accum) -> max -> argmax
            # -> per-tile gather.  4 tiles share one PSUM group + one reduce.
            GRP = 4
            for s in range(NB):
                zbh = zpool.tile([4, BLK], BF16, tag="zbh")
                nc.sync.dma_start(out=zbh[:], in_=z4h[:, s * BLK : (s + 1) * BLK])
                zbl = zpool.tile([4, BLK], BF16, tag="zbl")
                nc.sync.dma_start(out=zbl[:], in_=z4l[:, s * BLK : (s + 1) * BLK])
                # pixel(p; s, t) = s*BLK + p*TL + t
                zbh_r = zbh[:].rearrange("c (p t) -> c t p", t=TL)
                zbl_r = zbl[:].rearrange("c (p t) -> c t p", t=TL)
                for g in range(TL // GRP):
                    ps = pspool.tile([128, GRP * K], F32, tag="ps")
                    for u in range(GRP):
                        t = g * GRP + u
                        pv = ps[:, u * K : (u + 1) * K]
                        nc.tensor.matmul(
                            out=pv, lhsT=zbh_r[:, t, :], rhs=tba_h[:],
                            start=True, stop=False,
                        )
                        nc.tensor.matmul(
                            out=pv, lhsT=zbh_r[:, t, :], rhs=tba_l[:],
                            start=False, stop=False,
                        )
                        nc.tensor.matmul(
                            out=pv, lhsT=zbl_r[:, t, :], rhs=tba_h[:],
                            start=False, stop=True,
                        )
                    jg = s * TL + g * GRP
                    nc.vector.tensor_reduce(
                        out=smax_all[:, jg * 8 : (jg + GRP) * 8].rearrange(
                            "p (t e) -> p t e", e=8
                        )[:, :, 0],
                        in_=ps[:].rearrange("p (t k) -> p t k", k=K),
                        axis=mybir.AxisListType.X,
                        op=mybir.AluOpType.max,
                    )
                    for u in range(GRP):
                        j = jg + u
                        nc.vector.max_index(
                            idx_all[:, j * 8 : (j + 1) * 8],
                            smax_all[:, j * 8 : (j + 1) * 8],
                            ps[:, u * K : (u + 1) * K],
                        )
                        nc.gpsimd.indirect_dma_start(
                            out=zq4[:, 4 * j : 4 * (j + 1)],
                            out_offset=None,
                            in_=tb4[:],
                            in_offset=bass.IndirectOffsetOnAxis(
                                ap=idx_all[:, 8 * j : 8 * j + 1], axis=0
                            ),
                        )

            # sum of per-pixel max scores (col 0 of each 8-group), per partition
            smax_v = smax_all[:].rearrange("p (j e) -> p j e", e=8)[:, :, 0]
            scp = cpool.tile([128, NT], F32, tag="scp")
            nc.scalar.activation(
                out=scp[:], in_=smax_v,
                func=mybir.ActivationFunctionType.Copy,
                accum_out=stats[:, 0:1],
            )
            nc.sync.dma_start(out=st[:], in_=stats[:])

            # de-interleave channels and store output planes
            zq_v = zq4[:].rearrange("p (j c) -> p j c", c=4)
            for c in range(3):
                oc = outc[:, c * NT : (c + 1) * NT]
                nc.vector.tensor_copy(out=oc, in_=zq_v[:, :, c])
                nc.sync.dma_start(
                    out=o[c, :].rearrange("(s p t) -> p s t", s=NB, p=128, t=TL),
                    in_=oc.rearrange("p (s t) -> p s t", s=NB, t=TL),
                )
    return nc


_NC_CACHE = None
TRACE = False          # set True to capture an NTFF profile on the next call
LAST_EXEC_NS = None    # HW exec time from the last traced run
LAST_PROFILE = None    # profile json path/object from the last traced run


def _get_nc():
    global _NC_CACHE
    if _NC_CACHE is None:
        _NC_CACHE = build_kernel()
    return _NC_CACHE


def kernel(z: np.ndarray, color_table: np.ndarray):
    z = np.ascontiguousarray(np.asarray(z, dtype=np.float32))
    table = np.ascontiguousarray(np.asarray(color_table, dtype=np.float32))
    assert z.shape == (B, C, H, W) and table.shape == (K, 3)

    import ml_dtypes

    bf16 = ml_dtypes.bfloat16
    t2 = (table.astype(np.float64) ** 2).sum(-1).astype(np.float32)
    tba = np.concatenate([table.T, (-0.5 * t2)[None, :]], axis=0).astype(np.float32)
    tbah = tba.astype(bf16)
    tbal = (tba - tbah.astype(np.float32)).astype(bf16)
    tb4 = np.ascontiguousarray(
        np.concatenate([table, np.zeros((K, 1), np.float32)], axis=1)
    )

    ones = np.ones((1, N), np.float32)
    in_maps = []
    for b in range(B):
        z4 = np.ascontiguousarray(
            np.concatenate([z[b].reshape(3, N), ones], axis=0)
        )
        z4h = z4.astype(bf16)
        z4l = (z4 - z4h.astype(np.float32)).astype(bf16)
        in_maps.append(
            {"z4": z4, "z4h": z4h, "z4l": z4l,
             "tbah": tbah, "tbal": tbal, "tb4": tb4}
        )

    nc = _get_nc()
    global LAST_EXEC_NS, LAST_PROFILE
    res = run_bass_kernel_spmd(nc, in_maps, list(range(B)), trace=TRACE)
    if TRACE:
        LAST_EXEC_NS = res.exec_time_ns
        LAST_PROFILE = res.profile_json

    out = np.empty((B, C, H, W), np.float32)
    d2sum = 0.0
    for b in range(B):
        out[b] = res.results[b]["o"].reshape(3, H, W)
        stb = res.results[b]["st"].astype(np.float64)
        d2sum += stb[:, 1].sum() - 2.0 * stb[:, 0].sum()
    loss = np.float32((BETA + 1.0) * d2sum / (3.0 * B * N))
    return out, loss


if __name__ == "__main__":
    rng = np.random.default_rng(0)
    z = rng.standard_normal((B, C, H, W)).astype(np.float32)
    tbl = rng.random((K, 3), dtype=np.float32)
    out, loss = kernel(z=z, color_table=tbl)
    print("out", out.shape, "loss", loss)


# revision 13
# speedup vs baseline: 1.4368x; 1.4121x over previous
"""VQ color-lookup kernel for Trainium2 (8 NeuronCores, data-parallel over batch).

Problem: z [8,3,256,256] f32, color_table [512,3] f32.
  zl = z transposed to [B,H,W,3]; nearest color (squared L2) per pixel;
  out = quantized colors back in [B,3,H,W]; loss = 11*mean((z_q-zl)^2).

Per-core work (core b handles batch item b, N=65536 pixels):
  scores s[n,k] = <z_n, t_k> - 0.5*|t_k|^2  (argmax_k s = argmin_k d^2)
  computed by the PE as one K=4 fp32 matmul per 128-pixel tile
  (4th contraction row = ones against the -0.5*|t|^2 bias row).
  DVE max8 + max_index give the winning score + color index per pixel.
  z_q rows are fetched with one small indirect-DMA gather per tile
  (128 rows x 16B), overlapped with the DVE-bound main loop.
  Loss uses d2_min = |z|^2 - 2*s_max, accumulated per partition on ACT.
"""
import numpy as np

import concourse.bass as bass
import concourse.mybir as mybir
from concourse.tile import TileContext
from concourse.vector_clock import ScopedClock
from concourse.bass_utils import run_bass_kernel_spmd

B, C, H, W = 8, 3, 256, 256
N = H * W            # pixels per core (batch-sharded, one image per core)
K = 512              # colors
NB = 8               # super-blocks per core
TL = 64              # pixel tiles per super-block
BLK = N // NB        # pixels per super-block (8192)
NT = NB * TL         # total 128-pixel tiles per core (512)
F32 = mybir.dt.float32
BETA = 10.0


class TC(TileContext):
    """TileContext that splits sem waits one-per-instruction
    (this walrus build rejects >1 sync wait on a single instruction)."""

    def _split_waits(self, inst):
        si = getattr(inst, "sync_info", None)
        if not si or not si.on_wait or len(si.on_wait) <= 1:
            return
        if inst.engine == mybir.EngineType.Unassigned:
            return
        waits = list(si.on_wait)
        inst.sync_info = mybir.SyncInfo(
            on_wait=[waits[-1]], on_update=list(si.on_update or [])
        )
        for w in waits[:-1]:
            nop = mybir.InstNoOp(
                name=self.nc.get_next_instruction_name(), ins=[], outs=[]
            )
            nop.engine = inst.engine
            nop.sync_info = mybir.SyncInfo(on_wait=[w], on_update=[])
            self._add_instruction(nop)

    def _commit_instruction(self, inst, lazy_reg_writes: bool = True):
        self._split_waits(inst)
        super()._commit_instruction(inst, lazy_reg_writes)

    def _drain_and_barrier(self, tick_clock, wait_clock):
        nc = self.nc
        drain_inst = nc.sync.drain()
        wait_clock.add_sem_waits(
            drain_inst.ins, ScopedClock({None: tick_clock.global_clock})
        )
        si = drain_inst.ins.sync_info
        waits = list(si.on_wait) if si and si.on_wait else []
        if len(waits) > 1:
            drain_inst.ins.sync_info = mybir.SyncInfo(on_wait=[waits[0]], on_update=[])
            for w in waits[1:]:
                nop = nc.sync.nop(nofuse=True, hint="drain_split")
                nop.ins.sync_info = mybir.SyncInfo(on_wait=[w], on_update=[])
        nc.all_engine_barrier()
        assert self.sems is not None
        popped = nc._tile_sem_poison_stack.pop()
        assert popped is self._sem_poison
        nc.clear_and_free_semaphores(list(self.sems.allocated().values()))
        nc.all_engine_barrier()


def build_kernel():
    BF16 = mybir.dt.bfloat16
    nc = bass.Bass("TRN2", target_bir_lowering=False, debug=False, num_devices=8)
    z4 = nc.declare_dram_parameter("z4", [4, N], F32, isOutput=False)
    z4h = nc.declare_dram_parameter("z4h", [4, N], BF16, isOutput=False)
    z4l = nc.declare_dram_parameter("z4l", [4, N], BF16, isOutput=False)
    tbah = nc.declare_dram_parameter("tbah", [4, K], BF16, isOutput=False)
    tbal = nc.declare_dram_parameter("tbal", [4, K], BF16, isOutput=False)
    tb4 = nc.declare_dram_parameter("tb4", [K, 4], F32, isOutput=False)
    o = nc.declare_dram_parameter("o", [3, N], F32, isOutput=True)
    st = nc.declare_dram_parameter("st", [128, 2], F32, isOutput=True)

    with TC(nc) as tc:
        with (
            tc.tile_pool(name="const", bufs=1) as cpool,
            tc.tile_pool(name="zin", bufs=2) as zpool,
            tc.tile_pool(name="psum", bufs=4, space="PSUM") as pspool,
        ):
            tba_h = cpool.tile([4, K], BF16, tag="tbah")
            nc.sync.dma_start(out=tba_h[:], in_=tbah[:])
            tba_l = cpool.tile([4, K], BF16, tag="tbal")
            nc.sync.dma_start(out=tba_l[:], in_=tbal[:])

            smax_all = cpool.tile([128, 8 * NT], F32, tag="smax")
            idx_all = cpool.tile([128, 8 * NT], mybir.dt.uint32, tag="idx")
            stats = cpool.tile([128, 2], F32, tag="stats")
            zq4 = cpool.tile([128, 4 * NT], F32, tag="zq4")
            outc = cpool.tile([128, 3 * NT], F32, tag="outc")

            # |z|^2 per-partition sums: z4 rows 0..2 flat = 196608 = 128*1536
            zsq = cpool.tile([128, 1536], F32, tag="zsq")
            sqd = cpool.tile([128, 1536], F32, tag="sqd")
            nc.sync.dma_start(
                out=zsq[:],
                in_=z4[0:3, :].rearrange("c n -> (c n)").rearrange(
                    "(p f) -> p f", p=128
                ),
            )
            nc.scalar.activation(
                out=sqd[:], in_=zsq[:],
                func=mybir.ActivationFunctionType.Square,
                accum_out=stats[:, 1:2],
            )

            # main loop: scores (bf16 hi/lo split, fp32 accum) -> max -> argmax
            # -> per-tile gather.  4 tiles share one PSUM group + one reduce.
            GRP = 2
            for s in range(NB):
                zbh = zpool.tile([4, BLK], BF16, tag="zbh")
                nc.sync.dma_start(out=zbh[:], in_=z4h[:, s * BLK : (s + 1) * BLK])
                zbl = zpool.tile([4, BLK], BF16, tag="zbl")
                nc.sync.dma_start(out=zbl[:], in_=z4l[:, s * BLK : (s + 1) * BLK])
                # pixel(p; s, t) = s*BLK + p*TL + t
                zbh_r = zbh[:].rearrange("c (p t) -> c t p", t=TL)
                zbl_r = zbl[:].rearrange("c (p t) -> c t p", t=TL)
                for g in range(TL // GRP):
                    ps = pspool.tile([128, GRP * K], F32, tag="ps")
                    for u in range(GRP):
                        t = g * GRP + u
                        pv = ps[:, u * K : (u + 1) * K]
                        nc.tensor.matmul(
                            out=pv, lhsT=zbh_r[:, t, :], rhs=tba_h[:],
                            start=True, stop=False,
                        )
                        nc.tensor.matmul(
                            out=pv, lhsT=zbh_r[:, t, :], rhs=tba_l[:],
                            start=False, stop=False,
                        )
                        nc.tensor.matmul(
                            out=pv, lhsT=zbl_r[:, t, :], rhs=tba_h[:],
                            start=False, stop=True,
                        )
                    jg = s * TL + g * GRP
                    nc.vector.tensor_reduce(
                        out=smax_all[:, jg * 8 : (jg + GRP) * 8].rearrange(
                            "p (t e) -> p t e", e=8
                        )[:, :, 0],
                        in_=ps[:].rearrange("p (t k) -> p t k", k=K),
                        axis=mybir.AxisListType.X,
                        op=mybir.AluOpType.max,
                    )
                    for u in range(GRP):
                        j = jg + u
                        nc.vector.max_index(
                            idx_all[:, j * 8 : (j + 1) * 8],
                            smax_all[:, j * 8 : (j + 1) * 8],
                            ps[:, u * K : (u + 1) * K],
                        )
                        nc.gpsimd.indirect_dma_start(
                            out=zq4[:, 4 * j : 4 * (j + 1)],
                            out_offset=None,
                            in_=tb4[:],
                            in_offset=bass.IndirectOffsetOnAxis(
                                ap=idx_all[:, 8 * j : 8 * j + 1], axis=0
                            ),
                        )

            # sum of per-pixel max scores (col 0 of each 8-group), per partition
            smax_v = smax_all[:].rearrange("p (j e) -> p j e", e=8)[:, :, 0]
            scp = cpool.tile([128, NT], F32, tag="scp")
            nc.scalar.activation(
                out=scp[:], in_=smax_v,
                func=mybir.ActivationFunctionType.Copy,
                accum_out=stats[:, 0:1],
            )
            nc.sync.dma_start(out=st[:], in_=stats[:])

            # de-interleave channels and store output planes
            zq_v = zq4[:].rearrange("p (j c) -> p j c", c=4)
            for c in range(3):
                oc = outc[:, c * NT : (c + 1) * NT]
                nc.vector.tensor_copy(out=oc, in_=zq_v[:, :, c])
                nc.sync.dma_start(
                    out=o[c, :].rearrange("(s p t) -> p s t", s=NB, p=128, t=TL),
                    in_=oc.rearrange("p (s t) -> p s t", s=NB, t=TL),
                )
    return nc


_NC_CACHE = None
TRACE = False          # set True to capture an NTFF profile on the next call
LAST_EXEC_NS = None    # HW exec time from the last traced run
LAST_PROFILE = None    # profile json path/object from the last traced run


def _get_nc():
    global _NC_CACHE
    if _NC_CACHE is None:
        _NC_CACHE = build_kernel()
    return _NC_CACHE


def kernel(z: np.ndarray, color_table: np.ndarray):
    z = np.ascontiguousarray(np.asarray(z, dtype=np.float32))
    table = np.ascontiguousarray(np.asarray(color_table, dtype=np.float32))
    assert z.shape == (B, C, H, W) and table.shape == (K, 3)

    import ml_dtypes

    bf16 = ml_dtypes.bfloat16
    t2 = (table.astype(np.float64) ** 2).sum(-1).astype(np.float32)
    tba = np.concatenate([table.T, (-0.5 * t2)[None, :]], axis=0).astype(np.float32)
    tbah = tba.astype(bf16)
    tbal = (tba - tbah.astype(np.float32)).astype(bf16)
    tb4 = np.ascontiguousarray(
        np.concatenate([table, np.zeros((K, 1), np.float32)], axis=1)
    )

    ones = np.ones((1, N), np.float32)
    in_maps = []
    for b in range(B):
        z4 = np.ascontiguousarray(
            np.concatenate([z[b].reshape(3, N), ones], axis=0)
        )
        z4h = z4.astype(bf16)
        z4l = (z4 - z4h.astype(np.float32)).astype(bf16)
        in_maps.append(
            {"z4": z4, "z4h": z4h, "z4l": z4l,
             "tbah": tbah, "tbal": tbal, "tb4": tb4}
        )

    nc = _get_nc()
    global LAST_EXEC_NS, LAST_PROFILE
    res = run_bass_kernel_spmd(nc, in_maps, list(range(B)), trace=TRACE)
    if TRACE:
        LAST_EXEC_NS = res.exec_time_ns
        LAST_PROFILE = res.profile_json

    out = np.empty((B, C, H, W), np.float32)
    d2sum = 0.0
    for b in range(B):
        out[b] = res.results[b]["o"].reshape(3, H, W)
        stb = res.results[b]["st"].astype(np.float64)
        d2sum += stb[:, 1].sum() - 2.0 * stb[:, 0].sum()
    loss = np.float32((BETA + 1.0) * d2sum / (3.0 * B * N))
    return out, loss


if __name__ == "__main__":
    rng = np.random.default_rng(0)
    z = rng.standard_normal((B, C, H, W)).astype(np.float32)
    tbl = rng.random((K, 3), dtype=np.float32)
    out, loss = kernel(z=z, color_table=tbl)
    print("out", out.shape, "loss", loss)


# revision 18
# speedup vs baseline: 1.4463x; 1.0066x over previous
"""VQ color-lookup kernel for Trainium2 (8 NeuronCores, data-parallel over batch).

Problem: z [8,3,256,256] f32, color_table [512,3] f32.
  zl = z transposed to [B,H,W,3]; nearest color (squared L2) per pixel;
  out = quantized colors back in [B,3,H,W]; loss = 11*mean((z_q-zl)^2).

Per-core work (core b handles batch item b, N=65536 pixels):
  scores s[n,k] = <z_n, t_k> - 0.5*|t_k|^2  (argmax_k s = argmin_k d^2)
  computed by the PE as one K=4 fp32 matmul per 128-pixel tile
  (4th contraction row = ones against the -0.5*|t|^2 bias row).
  DVE max8 + max_index give the winning score + color index per pixel.
  z_q rows are fetched with one small indirect-DMA gather per tile
  (128 rows x 16B), overlapped with the DVE-bound main loop.
  Loss uses d2_min = |z|^2 - 2*s_max, accumulated per partition on ACT.
"""
import numpy as np

import concourse.bass as bass
import concourse.mybir as mybir
from concourse.tile import TileContext
from concourse.vector_clock import ScopedClock
from concourse.bass_utils import run_bass_kernel_spmd

B, C, H, W = 8, 3, 256, 256
N = H * W            # pixels per core (batch-sharded, one image per core)
K = 512              # colors
NB = 8               # super-blocks per core
TL = 64              # pixel tiles per super-block
BLK = N // NB        # pixels per super-block (8192)
NT = NB * TL         # total 128-pixel tiles per core (512)
F32 = mybir.dt.float32
BETA = 10.0


class TC(TileContext):
    """TileContext that splits sem waits one-per-instruction
    (this walrus build rejects >1 sync wait on a single instruction)."""

    def _split_waits(self, inst):
        si = getattr(inst, "sync_info", None)
        if not si or not si.on_wait or len(si.on_wait) <= 1:
            return
        if inst.engine == mybir.EngineType.Unassigned:
            return
        waits = list(si.on_wait)
        inst.sync_info = mybir.SyncInfo(
            on_wait=[waits[-1]], on_update=list(si.on_update or [])
        )
        for w in waits[:-1]:
            nop = mybir.InstNoOp(
                name=self.nc.get_next_instruction_name(), ins=[], outs=[]
            )
            nop.engine = inst.engine
            nop.sync_info = mybir.SyncInfo(on_wait=[w], on_update=[])
            self._add_instruction(nop)

    def _commit_instruction(self, inst, lazy_reg_writes: bool = True):
        self._split_waits(inst)
        super()._commit_instruction(inst, lazy_reg_writes)

    def _drain_and_barrier(self, tick_clock, wait_clock):
        nc = self.nc
        drain_inst = nc.sync.drain()
        wait_clock.add_sem_waits(
            drain_inst.ins, ScopedClock({None: tick_clock.global_clock})
        )
        si = drain_inst.ins.sync_info
        waits = list(si.on_wait) if si and si.on_wait else []
        if len(waits) > 1:
            drain_inst.ins.sync_info = mybir.SyncInfo(on_wait=[waits[0]], on_update=[])
            for w in waits[1:]:
                nop = nc.sync.nop(nofuse=True, hint="drain_split")
                nop.ins.sync_info = mybir.SyncInfo(on_wait=[w], on_update=[])
        nc.all_engine_barrier()
        assert self.sems is not None
        popped = nc._tile_sem_poison_stack.pop()
        assert popped is self._sem_poison
        nc.clear_and_free_semaphores(list(self.sems.allocated().values()))
        nc.all_engine_barrier()


def build_kernel():
    BF16 = mybir.dt.bfloat16
    nc = bass.Bass("TRN2", target_bir_lowering=False, debug=False, num_devices=8)
    z4 = nc.declare_dram_parameter("z4", [4, N], F32, isOutput=False)
    z12 = nc.declare_dram_parameter("z12", [12, N], BF16, isOutput=False)
    tbar = nc.declare_dram_parameter("tbar", [12, K], BF16, isOutput=False)
    tb4 = nc.declare_dram_parameter("tb4", [K, 4], F32, isOutput=False)
    o = nc.declare_dram_parameter("o", [3, N], F32, isOutput=True)
    st = nc.declare_dram_parameter("st", [128, 2], F32, isOutput=True)

    with TC(nc) as tc:
        with (
            tc.tile_pool(name="const", bufs=1) as cpool,
            tc.tile_pool(name="zin", bufs=2) as zpool,
            tc.tile_pool(name="psum", bufs=4, space="PSUM") as pspool,
        ):
            tba_r = cpool.tile([12, K], BF16, tag="tbar")
            nc.sync.dma_start(out=tba_r[:], in_=tbar[:])

            smax_all = cpool.tile([128, 8 * NT], F32, tag="smax")
            idx_all = cpool.tile([128, 8 * NT], mybir.dt.uint32, tag="idx")
            stats = cpool.tile([128, 2], F32, tag="stats")
            zq4 = cpool.tile([128, 4 * NT], F32, tag="zq4")
            outc = cpool.tile([128, 3 * NT], F32, tag="outc")

            # |z|^2 per-partition sums: z4 rows 0..2 flat = 196608 = 128*1536
            zsq = cpool.tile([128, 1536], F32, tag="zsq")
            sqd = cpool.tile([128, 1536], F32, tag="sqd")
            nc.sync.dma_start(
                out=zsq[:],
                in_=z4[0:3, :].rearrange("c n -> (c n)").rearrange(
                    "(p f) -> p f", p=128
                ),
            )
            nc.scalar.activation(
                out=sqd[:], in_=zsq[:],
                func=mybir.ActivationFunctionType.Square,
                accum_out=stats[:, 1:2],
            )

            # main loop: scores (bf16 hi/lo split, fp32 accum) -> max -> argmax
            # -> per-tile gather.  4 tiles share one PSUM group + one reduce.
            GRP = 2
            for s in range(NB):
                zb = zpool.tile([12, BLK], BF16, tag="zb")
                nc.sync.dma_start(out=zb[:], in_=z12[:, s * BLK : (s + 1) * BLK])
                # pixel(p; s, t) = s*BLK + p*TL + t
                zb_r = zb[:].rearrange("c (p t) -> c t p", t=TL)
                for g in range(TL // GRP):
                    ps = pspool.tile([128, GRP * K], F32, tag="ps")
                    for u in range(GRP):
                        t = g * GRP + u
                        pv = ps[:, u * K : (u + 1) * K]
                        nc.tensor.matmul(
                            out=pv, lhsT=zb_r[:, t, :], rhs=tba_r[:],
                            start=True, stop=True,
                        )
                    jg = s * TL + g * GRP
                    nc.vector.tensor_reduce(
                        out=smax_all[:, jg * 8 : (jg + GRP) * 8].rearrange(
                            "p (t e) -> p t e", e=8
                        )[:, :, 0],
                        in_=ps[:].rearrange("p (t k) -> p t k", k=K),
                        axis=mybir.AxisListType.X,
                        op=mybir.AluOpType.max,
                    )
                    for u in range(GRP):
                        j = jg + u
                        nc.vector.max_index(
                            idx_all[:, j * 8 : (j + 1) * 8],
                            smax_all[:, j * 8 : (j + 1) * 8],
                            ps[:, u * K : (u + 1) * K],
                        )
                        nc.gpsimd.indirect_dma_start(
                            out=zq4[:, 4 * j : 4 * (j + 1)],
                            out_offset=None,
                            in_=tb4[:],
                            in_offset=bass.IndirectOffsetOnAxis(
                                ap=idx_all[:, 8 * j : 8 * j + 1], axis=0
                            ),
                        )

            # sum of per-pixel max scores (col 0 of each 8-group), per partition
            smax_v = smax_all[:].rearrange("p (j e) -> p j e", e=8)[:, :, 0]
            scp = cpool.tile([128, NT], F32, tag="scp")
            nc.scalar.activation(
                out=scp[:], in_=smax_v,
                func=mybir.ActivationFunctionType.Copy,
                accum_out=stats[:, 0:1],
            )
            nc.sync.dma_start(out=st[:], in_=stats[:])

            # de-interleave channels and store output planes
            zq_v = zq4[:].rearrange("p (j c) -> p j c", c=4)
            for c in range(3):
                oc = outc[:, c * NT : (c + 1) * NT]
                nc.vector.tensor_copy(out=oc, in_=zq_v[:, :, c])
                nc.sync.dma_start(
                    out=o[c, :].rearrange("(s p t) -> p s t", s=NB, p=128, t=TL),
                    in_=oc.rearrange("p (s t) -> p s t", s=NB, t=TL),
                )
    return nc


_NC_CACHE = None
TRACE = False          # set True to capture an NTFF profile on the next call
LAST_EXEC_NS = None    # HW exec time from the last traced run
LAST_PROFILE = None    # profile json path/object from the last traced run


def _get_nc():
    global _NC_CACHE
    if _NC_CACHE is None:
        _NC_CACHE = build_kernel()
    return _NC_CACHE


def kernel(z: np.ndarray, color_table: np.ndarray):
    z = np.ascontiguousarray(np.asarray(z, dtype=np.float32))
    table = np.ascontiguousarray(np.asarray(color_table, dtype=np.float32))
    assert z.shape == (B, C, H, W) and table.shape == (K, 3)

    import ml_dtypes

    bf16 = ml_dtypes.bfloat16
    t2 = (table.astype(np.float64) ** 2).sum(-1).astype(np.float32)
    tba = np.concatenate([table.T, (-0.5 * t2)[None, :]], axis=0).astype(np.float32)
    tbah = tba.astype(bf16)
    tbal = (tba - tbah.astype(np.float32)).astype(bf16)
    tbar = np.ascontiguousarray(np.concatenate([tbah, tbal, tbah], axis=0))
    tb4 = np.ascontiguousarray(
        np.concatenate([table, np.zeros((K, 1), np.float32)], axis=1)
    )

    ones = np.ones((1, N), np.float32)
    in_maps = []
    for b in range(B):
        z4 = np.ascontiguousarray(
            np.concatenate([z[b].reshape(3, N), ones], axis=0)
        )
        z4h = z4.astype(bf16)
        z4l = (z4 - z4h.astype(np.float32)).astype(bf16)
        z12 = np.ascontiguousarray(np.concatenate([z4h, z4h, z4l], axis=0))
        in_maps.append({"z4": z4, "z12": z12, "tbar": tbar, "tb4": tb4})

    nc = _get_nc()
    global LAST_EXEC_NS, LAST_PROFILE
    res = run_bass_kernel_spmd(nc, in_maps, list(range(B)), trace=TRACE)
    if TRACE:
        LAST_EXEC_NS = res.exec_time_ns
        LAST_PROFILE = res.profile_json

    out = np.empty((B, C, H, W), np.float32)
    d2sum = 0.0
    for b in range(B):
        out[b] = res.results[b]["o"].reshape(3, H, W)
        stb = res.results[b]["st"].astype(np.float64)
        d2sum += stb[:, 1].sum() - 2.0 * stb[:, 0].sum()
    loss = np.float32((BETA + 1.0) * d2sum / (3.0 * B * N))
    return out, loss


if __name__ == "__main__":
    rng = np.random.default_rng(0)
    z = rng.standard_normal((B, C, H, W)).astype(np.float32)
    tbl = rng.random((K, 3), dtype=np.float32)
    out, loss = kernel(z=z, color_table=tbl)
    print("out", out.shape, "loss", loss)
